# revision 3
# baseline (speedup 1.0000x reference)
"""Trainium kernel for nn_NewLawinHead (LawinASPP segmentation head).

Contract: kernel(**inputs) takes FULL unsharded inputs (f1,f2,f3,f4,params)
and returns the FULL (2,19,64,64) float32 output.

Sharding strategy: data-parallel over batch B=2 (per the sharding hint);
each batch's forward is dispatched independently so the two halves run
concurrently. All weights are replicated. Device offload of the heavy
matmul pipeline is attempted per-batch on the NeuronCores when available;
the math below is an exact reimplementation of the reference network
(1x1 convs + eval-BN + ReLU, windowed pooled cross-attention at ratios
8/4/2, bilinear resizes) and falls back to host execution if device
compilation is unavailable, so the returned output is always correct.
"""

import numpy as np

EPS_BN = 1e-5
EPS_LN = 1e-5
RATIOS = [8, 4, 2]
HEADS = {8: 64, 4: 16, 2: 4}
D = 512
NUM_CLASSES = 19


def _forward_np(f1, f2, f3, f4, P, jnp, jax):
    def conv_bn_relu(x, w, g, b):
        y = jnp.einsum('bchw,oc->bohw', x, w)
        y = y * (g / jnp.sqrt(1.0 + EPS_BN))[:, None, None] + b[:, None, None]
        return jnp.maximum(y, 0.0)

    def mlp(x, w, b):
        return jnp.einsum('bchw,oc->bohw', x, w) + b[:, None, None]

    def layernorm_c(x, g, b):
        xt = jnp.moveaxis(x, 1, -1)
        mu = xt.mean(-1, keepdims=True)
        var = xt.var(-1, keepdims=True)
        xt = (xt - mu) / jnp.sqrt(var + EPS_LN) * g + b
        return jnp.moveaxis(xt, -1, 1)

    def resize(x, hw):
        B, C = x.shape[:2]
        return jax.image.resize(x, (B, C, hw[0], hw[1]), method='bilinear')

    def unfold(x, K, stride, pad):
        B, C, H, W = x.shape
        xp = jnp.pad(x, ((0, 0), (0, 0), (pad, pad), (pad, pad)))
        nh = (H + 2 * pad - K) // stride + 1
        nw = (W + 2 * pad - K) // stride + 1
        ri = (jnp.arange(nh) * stride)[:, None] + jnp.arange(K)[None, :]
        ci = (jnp.arange(nw) * stride)[:, None] + jnp.arange(K)[None, :]
        t = xp[:, :, ri, :]
        t = t[:, :, :, :, ci]
        t = t.transpose(0, 2, 4, 1, 3, 5).reshape(B * nh * nw, C, K, K)
        return t, nh, nw

    def patch_embed_pool(x, r, g, b):
        B, C, H, W = x.shape
        xr = x.reshape(B, C, H // r, r, W // r, r)
        p = 0.5 * (xr.max(axis=(3, 5)) + xr.mean(axis=(3, 5)))
        return layernorm_c(p, g, b)

    def attn(qx, cx, heads, qw, kvw, pw, pb):
        Bp, C, ph, _ = qx.shape
        hd = C // heads
        scale = hd ** -0.5
        qt = qx.reshape(Bp, C, ph * ph).transpose(0, 2, 1)
        ct = cx.reshape(Bp, C, -1).transpose(0, 2, 1)
        q = (qt @ qw.T).reshape(Bp, -1, heads, hd).transpose(0, 2, 1, 3)
        kv = (ct @ kvw.T).reshape(Bp, -1, 2, heads, hd).transpose(2, 0, 3, 1, 4)
        k, v = kv[0], kv[1]
        s = jnp.einsum('bhqd,bhkd->bhqk', q, k) * scale
        a = jax.nn.softmax(s, axis=-1)
        o = jnp.einsum('bhqk,bhkd->bhqd', a, v).transpose(0, 2, 1, 3).reshape(Bp, -1, C)
        o = o @ pw.T + pb
        return o.transpose(0, 2, 1).reshape(Bp, C, ph, ph)

    B, _, H, W = f2.shape
    c2 = mlp(f2, P['c2_w'], P['c2_b'])
    c3 = resize(mlp(f3, P['c3_w'], P['c3_b']), (H, W))
    c4 = resize(mlp(f4, P['c4_w'], P['c4_b']), (H, W))
    feat = conv_bn_relu(jnp.concatenate([c4, c3, c2], 1),
                        P['fuse_w'], P['fuse_g'], P['fuse_b'])
    feat_short = conv_bn_relu(feat, P['short_w'], P['short_g'], P['short_b'])
    pool = conv_bn_relu(feat.mean(axis=(2, 3), keepdims=True),
                        P['pool_w'], P['pool_g'], P['pool_b'])
    feat_pool = jnp.broadcast_to(pool, feat.shape)
    ps = 8
    query, nh, nw = unfold(feat, ps, ps, 0)
    lawin = []
    for r in RATIOS:
        ctx, _, _ = unfold(feat, ps * r, ps, int((r - 1) / 2 * ps))
        ctx = patch_embed_pool(ctx, r, P['ds%d_g' % r], P['ds%d_b' % r])
        o = attn(query, ctx, HEADS[r], P['q%d_w' % r], P['kv%d_w' % r],
                 P['p%d_w' % r], P['p%d_b' % r])
        o = (o.reshape(B, nh, nw, -1, ps, ps)
              .transpose(0, 3, 1, 4, 2, 5)
              .reshape(B, -1, nh * ps, nw * ps))
        lawin.append(o)
    out = conv_bn_relu(jnp.concatenate([feat_short, feat_pool] + lawin, 1),
                       P['cat_w'], P['cat_g'], P['cat_b'])
    c1 = mlp(f1, P['c1_w'], P['c1_b'])
    out = resize(out, (f1.shape[2], f1.shape[3]))
    fused = conv_bn_relu(jnp.concatenate([out, c1], 1),
                         P['low_w'], P['low_g'], P['low_b'])
    seg = jnp.einsum('bchw,oc->bohw', fused, P['pred_w']) + P['pred_b'][:, None, None]
    return seg


_FWD_CACHE = {}


def _get_fwd(jax, jnp):
    if 'fwd' not in _FWD_CACHE:
        _FWD_CACHE['fwd'] = jax.jit(
            lambda a, b, c, d, Pp: _forward_np(a, b, c, d, Pp, jnp, jax))
    return _FWD_CACHE['fwd']


def _put_params(P, dev, jax):
    # Replicated weights are immutable across calls: keep them device-resident,
    # keyed by a content fingerprint, so repeat invocations skip the transfer.
    import hashlib
    h = hashlib.sha1()
    for k in sorted(P):
        h.update(k.encode())
        h.update(np.ascontiguousarray(P[k]).tobytes())
    key = (repr(dev), h.hexdigest())
    if key not in _FWD_CACHE:
        _FWD_CACHE[key] = jax.device_put(P, dev)
    return _FWD_CACHE[key]


def _run_device(f1, f2, f3, f4, P, jax, jnp):
    # Data-parallel over batch B across the NeuronCores: batch element i runs
    # on device i. Both dispatches are async under jit, so the two batch
    # halves execute concurrently; weights are replicated to each device.
    devs = [d for d in jax.devices() if d.platform != 'cpu']
    if not devs:
        raise RuntimeError('no accelerator devices')
    B = f1.shape[0]
    fwd = _get_fwd(jax, jnp)
    outs = []
    for i in range(B):
        dev = devs[i % len(devs)]
        Pd = _put_params(P, dev, jax)
        outs.append(fwd(jax.device_put(f1[i:i + 1], dev),
                        jax.device_put(f2[i:i + 1], dev),
                        jax.device_put(f3[i:i + 1], dev),
                        jax.device_put(f4[i:i + 1], dev),
                        Pd))
    return np.concatenate([np.asarray(o) for o in outs], axis=0)


def _run_cpu(f1, f2, f3, f4, P, jax, jnp):
    cpu = jax.devices('cpu')[0]
    with jax.default_device(cpu):
        fwd = jax.jit(lambda a, b, c, d: _forward_np(a, b, c, d, P, jnp, jax))
        outs = [fwd(f1[i:i + 1], f2[i:i + 1], f3[i:i + 1], f4[i:i + 1])
                for i in range(f1.shape[0])]
        return np.concatenate([np.asarray(o) for o in outs], axis=0)


def kernel(f1, f2, f3, f4, params):
    import jax
    import jax.numpy as jnp

    f1 = np.asarray(f1, dtype=np.float32)
    f2 = np.asarray(f2, dtype=np.float32)
    f3 = np.asarray(f3, dtype=np.float32)
    f4 = np.asarray(f4, dtype=np.float32)
    P = {k: np.asarray(v, dtype=np.float32) for k, v in params.items()}

    try:
        seg = _run_device(f1, f2, f3, f4, P, jax, jnp)
    except Exception:
        seg = _run_cpu(f1, f2, f3, f4, P, jax, jnp)
    return seg.astype(np.float32)
